# revision 7
# baseline (speedup 1.0000x reference)
"""Causal group-query attention on 8 Trainium2 NeuronCores.

Sharding: core c -> (batch b = c // 4, kv-group g = c % 4).
Each core owns batch element b, q-heads [4g, 4g+4) and kv-group g (n_rep = 4,
so those 4 q-heads attend to exactly kv-group g's k/v).  Every core computes
its partial o_proj output (contracting head-concat columns [512g, 512g+512)),
and the host sums the 4 partials per batch element (the "all-reduce after
o_proj" done host-side since we return full outputs anyway).

Per-core kernel (T=2048, D=2048, HS=128):
  phase A (per 512-wide t-block): stream x^T, compute Q^T/K^T/V^T projections
    on the PE (contract over D in 16 k-tiles), add bias on ACT, apply RoPE on
    DVE: qf = qraw*cos + shift64(qraw)*sin2 where sin2 has the lower half
    pre-negated on the host, so the rotate-half is two partition-offset
    multiplies plus an add (no PE matmul).  V^T is transposed back to [t, hs]
    tiles via PE transpose.
  phase B (same t-block as q-block jq), heads processed in pairs: for each
    causally-valid 128-wide k-tile i and head h: S^T[tk, tq] = k-tile
    stationary x q^T moving (N<=512), P^T = exp(S^T/sqrt(HS)) on ACT (no max
    subtraction needed: scores are O(3)), triangular mask multiply on the
    diagonal subtile, then O^T[hs, tq] += V-stationary @ P^T in PSUM.  The
    softmax denominator is accumulated OFF the PE: per head a running
    acc[tk_p, tq] += P^T tile on DVE (head 0 of the pair) or GpSimd (head 1),
    then a single ones^T @ acc matmul per (head, t-block) makes the [1, 512]
    denominator row (vs a PE matmul per k-tile).
  normalize: reciprocal_approx_fast on the [1,512] row (DVE), GpSimd
    partition_broadcast fans it to [128,512], multiply into O^T on DVE.
  phase C: o_proj out[tq, d] = sum_h (O^T_h columns)-stationary @ Wo^T_h
    moving (N=512), evacuate to bf16 and DMA out (host upcasts + sums).

All matmuls use float32r (full-rate fp32 mode, 1 cycle/row at N>=256); every
producer of an fp32r matmul operand emits float32r-typed output (walrus
requires "rounded to FP32r" inputs). fp32r matmul outputs must sit at PSUM
partition 0 (col-group 0).
"""

import math

import numpy as np

B, T, D = 2, 2048, 2048
N_HEAD, N_GROUP = 16, 4
HS = D // N_HEAD  # 128
N_REP = N_HEAD // N_GROUP  # 4
NH_C = N_HEAD // N_GROUP  # heads per core = 4
INV_SQRT_HS = 1.0 / math.sqrt(HS)

_NC_CACHE: dict = {}


def build_nc(t=T, mm_r=True, rope_dve=True, den_off=True, pbcast=True,
             out_bf16=True, mq_dma=True):
    """Build and compile the per-core Bass program. Returns the compiled nc."""
    import concourse.bass as bass  # noqa: F401
    import concourse.mybir as mybir
    import concourse.tile as tile
    from concourse import bacc

    f32 = mybir.dt.float32
    f32r = mybir.dt.float32r
    bf16 = mybir.dt.bfloat16
    ident_f = mybir.ActivationFunctionType.Identity
    exp_f = mybir.ActivationFunctionType.Exp

    def r(ap):
        # float32r view: fp32 matmul at full issue rate (1 cycle/row, N>=256)
        return ap.bitcast(f32r) if mm_r else ap

    nd = D // 128  # d-tiles (contraction) = 16
    tb_n = t // 512  # 512-wide t blocks
    nk = t // 128  # 128-wide k tiles

    nc = bacc.Bacc("TRN2", target_bir_lowering=False, debug=False)

    xd = nc.dram_tensor("x_t", [D, t], f32, kind="ExternalInput")
    wqd = nc.dram_tensor("wq_t", [D, NH_C * HS], f32, kind="ExternalInput")
    wkd = nc.dram_tensor("wk_t", [D, HS], f32, kind="ExternalInput")
    wvd = nc.dram_tensor("wv_t", [D, HS], f32, kind="ExternalInput")
    wod = nc.dram_tensor("wo_t", [NH_C * HS, D], f32, kind="ExternalInput")
    cosd = nc.dram_tensor("cos_t", [HS, t], f32, kind="ExternalInput")
    # sin2: lower 64 partitions pre-negated on host (rotate-half sign)
    sind = nc.dram_tensor("sin_t", [HS, t], f32, kind="ExternalInput")
    bqd = nc.dram_tensor("b_q", [HS, NH_C], f32, kind="ExternalInput")
    bkd = nc.dram_tensor("b_k", [HS, 1], f32, kind="ExternalInput")
    bvd = nc.dram_tensor("b_v", [HS, 1], f32, kind="ExternalInput")
    maskd = nc.dram_tensor("mask_ut", [128, 128], f32, kind="ExternalInput")
    identd = nc.dram_tensor("ident", [128, 128], f32, kind="ExternalInput")
    out_dt = bf16 if out_bf16 else f32
    outd = nc.dram_tensor("out", [t, D], out_dt, kind="ExternalOutput")

    with tile.TileContext(nc) as tc:
        with (
            tc.tile_pool(name="consts", bufs=1) as consts,
            tc.tile_pool(name="wpool", bufs=1) as wpool,
            tc.tile_pool(name="resid", bufs=1) as resid,
            tc.tile_pool(name="xin", bufs=4) as xin,
            tc.tile_pool(name="work", bufs=3) as work,
            tc.tile_pool(name="qfp", bufs=2) as qfp,
            tc.tile_pool(name="otp", bufs=6) as otp,
            tc.tile_pool(name="outp", bufs=4) as outp,
            tc.tile_pool(name="accp", bufs=2) as accp,
            tc.tile_pool(name="psum", bufs=8, space="PSUM") as psum,
        ):
            def bank(name):
                return psum.tile([128, 512], f32, tag="bank", name=name)

            # ---- constants / weights (loaded once) ----
            cos_sb = consts.tile([128, t], f32, name="cos_sb")
            sin_sb = consts.tile([128, t], f32, name="sin_sb")
            mask_sb = consts.tile([128, 128], f32, name="mask_sb")
            id_sb = consts.tile([128, 128], f32, name="id_sb")
            ones_f = consts.tile([128, 128], f32, name="ones_f")
            ones_sb = consts.tile([128, 128], f32, name="ones_sb")
            bq_sb = consts.tile([128, NH_C], f32, name="bq_sb")
            bk_sb = consts.tile([128, 1], f32, name="bk_sb")
            bv_sb = consts.tile([128, 1], f32, name="bv_sb")
            wq_sb = wpool.tile([128, nd, NH_C * HS], f32, name="wq_sb")
            wk_sb = wpool.tile([128, nd, HS], f32, name="wk_sb")
            wv_sb = wpool.tile([128, nd, HS], f32, name="wv_sb")
            wo_sb = wpool.tile([128, NH_C, D], f32, name="wo_sb")
            wq_re = wqd[:, :].rearrange("(n p) m -> p n m", p=128)
            wk_re = wkd[:, :].rearrange("(n p) m -> p n m", p=128)
            wv_re = wvd[:, :].rearrange("(n p) m -> p n m", p=128)

            # resident K^T [hs, t] and V [t(128-tiles), hs]
            kt_sb = resid.tile([128, t], f32, name="kt_sb")
            v_sb = resid.tile([128, nk, HS], f32, name="v_sb")

            x_re = xd[:, :].rearrange("(n p) t -> p n t", p=128)

            # weight-stream queue: keep the sync queue exclusive to xt
            # (DMA-capable engines are sync, scalar/Activation, gpsimd)
            wq_q = nc.scalar if mq_dma else nc.sync
            wkv_q = nc.scalar if mq_dma else nc.sync

            def emit_oproj(tb, ot_sb):
                # o_proj partial for q-block tb; emitted one block late so the
                # PE fills the next block's RoPE-latency gap with these
                # matmuls while ACT/DVE produce qf.
                ts0 = tb * 512
                for s in range(4):
                    for db in range(D // 512):
                        op_ps = bank("op_ps")
                        for h in range(NH_C):
                            nc.tensor.matmul(
                                op_ps,
                                lhsT=r(ot_sb[h][:, 128 * s : 128 * (s + 1)]),
                                rhs=r(wo_sb[:, h, 512 * db : 512 * (db + 1)]),
                                start=h == 0, stop=h == NH_C - 1,
                            )
                        ob = outp.tile([128, 512], out_dt, name="ob")
                        if (s + db) % 2 == 0:
                            nc.vector.tensor_copy(out=ob, in_=op_ps)
                        else:
                            nc.scalar.copy(out=ob, in_=op_ps)
                        nc.gpsimd.dma_start(
                            out=outd[
                                ts0 + 128 * s : ts0 + 128 * (s + 1),
                                512 * db : 512 * (db + 1),
                            ],
                            in_=ob,
                        )

            def rope(dst, src, ts0):
                # dst = src*cos + rotate_half(src)*sin.  sin_sb holds the
                # halves swapped and sign-folded (sin_sw[p<64] = +sin[p+64],
                # sin_sw[p>=64] = -sin[p-64]) so each multiply reads both
                # SBUF inputs at the same base partition (walrus constraint);
                # only the destination partition base is shifted.
                nc.vector.tensor_mul(r(dst), src, cos_sb[:, ts0 : ts0 + 512])
                rt = work.tile([128, 512], f32, name="rtmp", bufs=2)
                nc.vector.tensor_mul(
                    rt[0:64, :], src[64:128, :], sin_sb[64:128, ts0 : ts0 + 512]
                )
                nc.vector.tensor_mul(
                    rt[64:128, :], src[0:64, :], sin_sb[0:64, ts0 : ts0 + 512]
                )
                nc.vector.tensor_add(r(dst), dst, rt)

            pending_oproj = None
            for tb in range(tb_n):
                ts0 = tb * 512
                # ============ phase A: projections + RoPE for this t-block
                qt_ps = [bank(f"qt_ps{h}") for h in range(NH_C)]
                kt_ps = bank("kt_ps")
                vt_ps = bank("vt_ps")
                for chunk in range(nd // 2):
                    c2 = 2 * chunk
                    xt = xin.tile([128, 2, 512], f32, name="xt")
                    nc.sync.dma_start(
                        out=r(xt),
                        in_=r(x_re[:, c2 : c2 + 2, ts0 : ts0 + 512]),
                    )
                    if tb == 0:
                        wq_q.dma_start(
                            out=r(wq_sb[:, c2 : c2 + 2, :]),
                            in_=r(wq_re[:, c2 : c2 + 2, :]),
                        )
                        wkv_q.dma_start(
                            out=r(wk_sb[:, c2 : c2 + 2, :]),
                            in_=r(wk_re[:, c2 : c2 + 2, :]),
                        )
                        wkv_q.dma_start(
                            out=r(wv_sb[:, c2 : c2 + 2, :]),
                            in_=r(wv_re[:, c2 : c2 + 2, :]),
                        )
                        if chunk == 0:
                            wkv_q.dma_start(out=bq_sb, in_=bqd[:, :])
                            wkv_q.dma_start(out=bk_sb, in_=bkd[:, :])
                            wkv_q.dma_start(out=bv_sb, in_=bvd[:, :])
                            nc.vector.memset(ones_f, 1.0)
                            nc.scalar.copy(out=r(ones_sb), in_=ones_f)
                    for j in range(2):
                        dt = c2 + j
                        first, last = dt == 0, dt == nd - 1
                        for h in range(NH_C):
                            nc.tensor.matmul(
                                qt_ps[h],
                                lhsT=r(wq_sb[:, dt, h * HS : (h + 1) * HS]),
                                rhs=r(xt[:, j, :]),
                                start=first,
                                stop=last,
                            )
                        nc.tensor.matmul(
                            kt_ps, lhsT=r(wk_sb[:, dt, :]), rhs=r(xt[:, j, :]),
                            start=first, stop=last,
                        )
                        nc.tensor.matmul(
                            vt_ps, lhsT=r(wv_sb[:, dt, :]), rhs=r(xt[:, j, :]),
                            start=first, stop=last,
                        )

                if tb == 0:
                    # one-time loads go on the gpsimd queue so the sync queue
                    # stays dedicated to the xt stream
                    nc.gpsimd.dma_start(out=cos_sb, in_=cosd[:, :])
                    nc.gpsimd.dma_start(out=sin_sb, in_=sind[:, :])
                    nc.gpsimd.dma_start(out=mask_sb, in_=maskd[:, :])
                    nc.gpsimd.dma_start(out=id_sb, in_=identd[:, :])
                    wo_re = wod[:, :].rearrange("(h p) m -> p h m", p=128)
                    for h in range(NH_C):
                        nc.gpsimd.dma_start(
                            out=r(wo_sb[:, h : h + 1, :]),
                            in_=r(wo_re[:, h : h + 1, :]),
                        )

                # q: bias + rope -> qf [128, h, 512]
                qf = qfp.tile([128, NH_C, 512], f32, name="qf")
                for h in range(NH_C):
                    qraw = work.tile([128, 512], f32, name="qraw")
                    nc.scalar.activation(
                        out=r(qraw), in_=qt_ps[h], func=ident_f,
                        bias=bq_sb[:, h : h + 1], scale=1.0,
                    )
                    rope(qf[:, h, :], qraw, ts0)

                # k: bias + rope -> kt_sb slice
                kraw = work.tile([128, 512], f32, name="qraw")
                nc.scalar.activation(
                    out=r(kraw), in_=kt_ps, func=ident_f, bias=bk_sb[:, 0:1],
                    scale=1.0,
                )
                rope(kt_sb[:, ts0 : ts0 + 512], kraw, ts0)

                # v: bias, then transpose to [t, hs] tiles
                vraw = work.tile([128, 512], f32, name="qraw")
                nc.scalar.activation(
                    out=vraw, in_=vt_ps, func=ident_f, bias=bv_sb[:, 0:1], scale=1.0
                )
                for s in range(4):
                    vt_tp = bank("vt_tp")
                    nc.tensor.transpose(
                        vt_tp[:, 0:128], vraw[:, 128 * s : 128 * (s + 1)], id_sb[:, :]
                    )
                    nc.scalar.copy(
                        out=r(v_sb[:, 4 * tb + s, :]), in_=vt_tp[:, 0:128]
                    )

                # previous block's o_proj: PE chews on it while ACT/DVE
                # finish this block's RoPE chain (qf not needed yet).
                if pending_oproj is not None:
                    emit_oproj(tb - 1, pending_oproj)

                # ============ phase B: attention for q-block jq == tb
                # Software-pipelined by one k-tile: PV of tile i-1 issues
                # while ACT computes exp of tile i, so the PE never waits on
                # the st->exp->mask chain.  The softmax denominator runs off
                # the PE: acc[h] += pt on DVE (pair head 0) / GpSimd (head 1).
                ot_sb = {}
                imax = 4 * tb + 3
                for hp in range(NH_C // 2):
                    heads = (2 * hp, 2 * hp + 1)
                    ot_ps = {h: bank(f"ot_ps{h}") for h in heads}
                    den_ps = {}
                    acc = {}
                    if den_off:
                        acc = {
                            h: accp.tile([128, 512], f32, name=f"acc{h & 1}")
                            for h in heads
                        }
                    else:
                        den_ps = {h: bank(f"den_ps{h}") for h in heads}

                    def emit_pv(i, pts, c0):
                        first, last = i == 0, i == imax
                        for h in heads:
                            nc.tensor.matmul(
                                ot_ps[h][:, c0:],
                                lhsT=r(v_sb[:, i, :]),
                                rhs=r(pts[h][:, c0:]),
                                start=first, stop=last,
                            )
                            if not den_off:
                                nc.tensor.matmul(
                                    den_ps[h][0:1, c0:],
                                    lhsT=r(ones_sb[:, 0:1]),
                                    rhs=r(pts[h][:, c0:]),
                                    start=first, stop=last,
                                    skip_group_check=True,
                                )

                    prev = None
                    for i in range(imax + 1):
                        c0 = 128 * max(0, i - 4 * tb)
                        diag = i >= 4 * tb
                        pts = {}
                        for h in heads:
                            st_ps = bank("st_ps")
                            nc.tensor.matmul(
                                st_ps[:, c0:],
                                lhsT=r(kt_sb[:, 128 * i : 128 * (i + 1)]),
                                rhs=r(qf[:, h, c0:]),
                                start=True, stop=True,
                            )
                            pt = work.tile([128, 512], f32, name="pt", bufs=4)
                            nc.scalar.activation(
                                out=r(pt[:, c0:]), in_=st_ps[:, c0:], func=exp_f,
                                scale=INV_SQRT_HS,
                            )
                            if diag:
                                nc.vector.tensor_mul(
                                    r(pt[:, c0 : c0 + 128]),
                                    pt[:, c0 : c0 + 128],
                                    mask_sb,
                                )
                            pts[h] = pt
                            if den_off:
                                # engine-local accumulation chain per head
                                eng = nc.vector if (h & 1) == 0 else nc.gpsimd
                                if i == 0:
                                    eng.tensor_copy(
                                        out=r(acc[h][:, c0:]), in_=pt[:, c0:]
                                    )
                                else:
                                    eng.tensor_add(
                                        r(acc[h][:, c0:]), acc[h][:, c0:], pt[:, c0:]
                                    )
                        if prev is not None:
                            emit_pv(*prev)
                        prev = (i, pts, c0)
                    emit_pv(*prev)

                    # normalize each head's O^T by its softmax denominator:
                    # one ones^T @ acc matmul makes the [1,512] denominator
                    # row; reciprocal_approx_fast on that row (DVE, ~18-bit),
                    # GpSimd partition_broadcast fans it out to [128,512], and
                    # DVE multiplies it into the evacuated O^T.  Evacuate O^T
                    # (ACT copies) immediately so PSUM banks free early.
                    osb_h = {}
                    denrow = {}
                    for h in heads:
                        osb = otp.tile([128, 512], f32, name="osb", bufs=8)
                        nc.scalar.copy(out=r(osb), in_=ot_ps[h])
                        osb_h[h] = osb
                        if den_off:
                            dps = bank("den_ps")
                            nc.tensor.matmul(
                                dps[0:1, :],
                                lhsT=r(ones_sb[:, 0:1]),
                                rhs=r(acc[h]),
                                start=True, stop=True,
                                skip_group_check=True,
                            )
                            den_ps[h] = dps
                        dr = work.tile([1, 512], f32, name="den_sb", bufs=2)
                        nc.scalar.copy(out=dr, in_=den_ps[h][0:1, :])
                        denrow[h] = dr
                    for h in heads:
                        bc_sb = work.tile([128, 512], f32, name="bc_sb", bufs=2)
                        if pbcast:
                            nc.vector.reciprocal_approx_fast(
                                out=denrow[h], in_=denrow[h]
                            )
                            nc.gpsimd.partition_broadcast(
                                bc_sb, denrow[h], channels=128
                            )
                        else:
                            bc_ps = bank("bc_ps")
                            nc.tensor.matmul(
                                bc_ps,
                                lhsT=r(ones_sb[0:1, 0:128]),
                                rhs=r(denrow[h]),
                                start=True, stop=True,
                            )
                            nc.scalar.copy(out=bc_sb, in_=bc_ps)
                            nc.vector.reciprocal_approx_fast(out=bc_sb, in_=bc_sb)
                        nc.vector.tensor_mul(r(osb_h[h]), osb_h[h], bc_sb)
                        ot_sb[h] = osb_h[h]

                pending_oproj = ot_sb

            emit_oproj(tb_n - 1, pending_oproj)

    nc.compile()
    return nc


def shard_inputs(x, cos, sin, Wq, bq, Wkv, bkv, Wo, t=T):
    """Build the 8 per-core input maps (core c -> batch c//4, group c%4)."""
    f32 = np.float32
    mask_ut = np.triu(np.ones((128, 128), f32))
    ident = np.eye(128, dtype=f32)
    cos_t = np.ascontiguousarray(cos.T.astype(f32))
    # sin_sw: halves swapped with the rotate-half sign folded in, so the
    # on-chip rotate is two same-base-partition multiplies plus an add:
    # sin_sw[p<64] = +sin^T[p+64], sin_sw[p>=64] = -sin^T[p-64].
    st = sin.T.astype(f32)
    sin_t = np.ascontiguousarray(np.concatenate([st[64:128], -st[0:64]], axis=0))

    xts = [np.ascontiguousarray(x[b].T.astype(f32)) for b in range(x.shape[0])]
    per_g = []
    for g in range(4):
        per_g.append(
            dict(
                wq_t=np.ascontiguousarray(Wq[512 * g : 512 * g + 512].T.astype(f32)),
                b_q=np.ascontiguousarray(
                    bq[512 * g : 512 * g + 512].reshape(4, 128).T.astype(f32)
                ),
                wk_t=np.ascontiguousarray(
                    Wkv[128 * g : 128 * g + 128].T.astype(f32)
                ),
                b_k=np.ascontiguousarray(
                    bkv[128 * g : 128 * g + 128].reshape(128, 1).astype(f32)
                ),
                wv_t=np.ascontiguousarray(
                    Wkv[512 + 128 * g : 512 + 128 * g + 128].T.astype(f32)
                ),
                b_v=np.ascontiguousarray(
                    bkv[512 + 128 * g : 512 + 128 * g + 128]
                    .reshape(128, 1)
                    .astype(f32)
                ),
                wo_t=np.ascontiguousarray(
                    Wo[:, 512 * g : 512 * g + 512].T.astype(f32)
                ),
            )
        )

    in_maps = []
    for c in range(4 * x.shape[0]):
        b, g = c // 4, c % 4
        m = dict(per_g[g])
        m.update(
            x_t=xts[b], cos_t=cos_t, sin_t=sin_t,
            mask_ut=mask_ut, ident=ident,
        )
        in_maps.append(m)
    return in_maps


def run_on_hw(in_maps, t=T, trace=False, mm_r=True, **flags):
    from concourse.bass_utils import run_bass_kernel_spmd

    key = (t, mm_r, tuple(sorted(flags.items())))
    if key not in _NC_CACHE:
        _NC_CACHE[key] = build_nc(t, mm_r=mm_r, **flags)
    nc = _NC_CACHE[key]
    res = run_bass_kernel_spmd(
        nc, in_maps, core_ids=list(range(len(in_maps))), trace=trace
    )
    return res


def kernel(x, cos, sin, Wq, bq, Wkv, bkv, Wo):
    x = np.asarray(x)
    in_maps = shard_inputs(
        x, np.asarray(cos), np.asarray(sin), np.asarray(Wq), np.asarray(bq),
        np.asarray(Wkv), np.asarray(bkv), np.asarray(Wo),
    )
    res = run_on_hw(in_maps, t=T, trace=False)
    out = np.zeros((B, T, D), np.float32)
    for c, rmap in enumerate(res.results):
        out[c // 4] += np.asarray(rmap["out"], dtype=np.float32)
    return out


# revision 8
# speedup vs baseline: 1.3068x; 1.3068x over previous
"""Causal group-query attention on 8 Trainium2 NeuronCores.

Sharding: core c -> (batch b = c // 4, kv-group g = c % 4).
Each core owns batch element b, q-heads [4g, 4g+4) and kv-group g (n_rep = 4,
so those 4 q-heads attend to exactly kv-group g's k/v).  Every core computes
its partial o_proj output (contracting head-concat columns [512g, 512g+512)),
and the host sums the 4 partials per batch element (the "all-reduce after
o_proj" done host-side since we return full outputs anyway).

Per-core kernel (T=2048, D=2048, HS=128):
  phase A (per 512-wide t-block): stream x^T, compute Q^T/K^T/V^T projections
    on the PE (contract over D in 16 k-tiles), add bias on ACT, apply RoPE on
    DVE: qf = qraw*cos + shift64(qraw)*sin2 where sin2 has the lower half
    pre-negated on the host, so the rotate-half is two partition-offset
    multiplies plus an add (no PE matmul).  V^T is transposed back to [t, hs]
    tiles via PE transpose.
  phase B (same t-block as q-block jq), heads processed in pairs: for each
    causally-valid 128-wide k-tile i and head h: S^T[tk, tq] = k-tile
    stationary x q^T moving (N<=512), P^T = exp(S^T/sqrt(HS)) on ACT (no max
    subtraction needed: scores are O(3)), triangular mask multiply on the
    diagonal subtile, then O^T[hs, tq] += V-stationary @ P^T in PSUM.  The
    softmax denominator is accumulated OFF the PE: per head a running
    acc[tk_p, tq] += P^T tile on DVE (head 0 of the pair) or GpSimd (head 1),
    then a single ones^T @ acc matmul per (head, t-block) makes the [1, 512]
    denominator row (vs a PE matmul per k-tile).
  normalize: reciprocal_approx_fast on the [1,512] row (DVE), GpSimd
    partition_broadcast fans it to [128,512], multiply into O^T on DVE.
  phase C: o_proj out[tq, d] = sum_h (O^T_h columns)-stationary @ Wo^T_h
    moving (N=512), evacuate to bf16 and DMA out (host upcasts + sums).

All matmuls use float32r (full-rate fp32 mode, 1 cycle/row at N>=256); every
producer of an fp32r matmul operand emits float32r-typed output (walrus
requires "rounded to FP32r" inputs). fp32r matmul outputs must sit at PSUM
partition 0 (col-group 0).
"""

import math

import numpy as np

B, T, D = 2, 2048, 2048
N_HEAD, N_GROUP = 16, 4
HS = D // N_HEAD  # 128
N_REP = N_HEAD // N_GROUP  # 4
NH_C = N_HEAD // N_GROUP  # heads per core = 4
INV_SQRT_HS = 1.0 / math.sqrt(HS)

_NC_CACHE: dict = {}


def build_nc(t=T, mm_r=True, rope_dve=True, den_off=False, pbcast=True,
             out_bf16=True, mq_dma=True):
    """Build and compile the per-core Bass program. Returns the compiled nc."""
    import concourse.bass as bass  # noqa: F401
    import concourse.mybir as mybir
    import concourse.tile as tile
    from concourse import bacc

    f32 = mybir.dt.float32
    f32r = mybir.dt.float32r
    bf16 = mybir.dt.bfloat16
    ident_f = mybir.ActivationFunctionType.Identity
    exp_f = mybir.ActivationFunctionType.Exp

    def r(ap):
        # float32r view: fp32 matmul at full issue rate (1 cycle/row, N>=256)
        return ap.bitcast(f32r) if mm_r else ap

    nd = D // 128  # d-tiles (contraction) = 16
    tb_n = t // 512  # 512-wide t blocks
    nk = t // 128  # 128-wide k tiles

    nc = bacc.Bacc("TRN2", target_bir_lowering=False, debug=False)

    xd = nc.dram_tensor("x_t", [D, t], f32, kind="ExternalInput")
    wqd = nc.dram_tensor("wq_t", [D, NH_C * HS], f32, kind="ExternalInput")
    wkd = nc.dram_tensor("wk_t", [D, HS], f32, kind="ExternalInput")
    wvd = nc.dram_tensor("wv_t", [D, HS], f32, kind="ExternalInput")
    wod = nc.dram_tensor("wo_t", [NH_C * HS, D], f32, kind="ExternalInput")
    cosd = nc.dram_tensor("cos_t", [HS, t], f32, kind="ExternalInput")
    # sin2: lower 64 partitions pre-negated on host (rotate-half sign)
    sind = nc.dram_tensor("sin_t", [HS, t], f32, kind="ExternalInput")
    bqd = nc.dram_tensor("b_q", [HS, NH_C], f32, kind="ExternalInput")
    bkd = nc.dram_tensor("b_k", [HS, 1], f32, kind="ExternalInput")
    bvd = nc.dram_tensor("b_v", [HS, 1], f32, kind="ExternalInput")
    maskd = nc.dram_tensor("mask_ut", [128, 128], f32, kind="ExternalInput")
    identd = nc.dram_tensor("ident", [128, 128], f32, kind="ExternalInput")
    out_dt = bf16 if out_bf16 else f32
    outd = nc.dram_tensor("out", [t, D], out_dt, kind="ExternalOutput")

    with tile.TileContext(nc) as tc:
        with (
            tc.tile_pool(name="consts", bufs=1) as consts,
            tc.tile_pool(name="wpool", bufs=1) as wpool,
            tc.tile_pool(name="resid", bufs=1) as resid,
            tc.tile_pool(name="xin", bufs=4) as xin,
            tc.tile_pool(name="work", bufs=3) as work,
            tc.tile_pool(name="qfp", bufs=2) as qfp,
            tc.tile_pool(name="otp", bufs=6) as otp,
            tc.tile_pool(name="outp", bufs=4) as outp,
            tc.tile_pool(name="accp", bufs=2) as accp,
            tc.tile_pool(name="psum", bufs=8, space="PSUM") as psum,
        ):
            def bank(name):
                return psum.tile([128, 512], f32, tag="bank", name=name)

            # ---- constants / weights (loaded once) ----
            cos_sb = consts.tile([128, t], f32, name="cos_sb")
            sin_sb = consts.tile([128, t], f32, name="sin_sb")
            mask_sb = consts.tile([128, 128], f32, name="mask_sb")
            id_sb = consts.tile([128, 128], f32, name="id_sb")
            ones_f = consts.tile([128, 128], f32, name="ones_f")
            ones_sb = consts.tile([128, 128], f32, name="ones_sb")
            bq_sb = consts.tile([128, NH_C], f32, name="bq_sb")
            bk_sb = consts.tile([128, 1], f32, name="bk_sb")
            bv_sb = consts.tile([128, 1], f32, name="bv_sb")
            wq_sb = wpool.tile([128, nd, NH_C * HS], f32, name="wq_sb")
            wk_sb = wpool.tile([128, nd, HS], f32, name="wk_sb")
            wv_sb = wpool.tile([128, nd, HS], f32, name="wv_sb")
            wo_sb = wpool.tile([128, NH_C, D], f32, name="wo_sb")
            wq_re = wqd[:, :].rearrange("(n p) m -> p n m", p=128)
            wk_re = wkd[:, :].rearrange("(n p) m -> p n m", p=128)
            wv_re = wvd[:, :].rearrange("(n p) m -> p n m", p=128)

            # resident K^T [hs, t] and V [t(128-tiles), hs]
            kt_sb = resid.tile([128, t], f32, name="kt_sb")
            v_sb = resid.tile([128, nk, HS], f32, name="v_sb")

            x_re = xd[:, :].rearrange("(n p) t -> p n t", p=128)

            # weight-stream queue: keep the sync queue exclusive to xt
            # (DMA-capable engines are sync, scalar/Activation, gpsimd)
            wq_q = nc.scalar if mq_dma else nc.sync
            wkv_q = nc.scalar if mq_dma else nc.sync

            def emit_oproj(tb, ot_sb):
                # o_proj partial for q-block tb; emitted one block late so the
                # PE fills the next block's RoPE-latency gap with these
                # matmuls while ACT/DVE produce qf.
                ts0 = tb * 512
                for s in range(4):
                    for db in range(D // 512):
                        op_ps = bank("op_ps")
                        for h in range(NH_C):
                            nc.tensor.matmul(
                                op_ps,
                                lhsT=r(ot_sb[h][:, 128 * s : 128 * (s + 1)]),
                                rhs=r(wo_sb[:, h, 512 * db : 512 * (db + 1)]),
                                start=h == 0, stop=h == NH_C - 1,
                            )
                        ob = outp.tile([128, 512], out_dt, name="ob")
                        if (s + db) % 2 == 0:
                            nc.vector.tensor_copy(out=ob, in_=op_ps)
                        else:
                            nc.scalar.copy(out=ob, in_=op_ps)
                        nc.gpsimd.dma_start(
                            out=outd[
                                ts0 + 128 * s : ts0 + 128 * (s + 1),
                                512 * db : 512 * (db + 1),
                            ],
                            in_=ob,
                        )

            def rope(dst, src, ts0):
                # dst = src*cos + rotate_half(src)*sin.  sin_sb holds the
                # halves swapped and sign-folded (sin_sw[p<64] = +sin[p+64],
                # sin_sw[p>=64] = -sin[p-64]) so each multiply reads both
                # SBUF inputs at the same base partition (walrus constraint);
                # only the destination partition base is shifted.
                nc.vector.tensor_mul(r(dst), src, cos_sb[:, ts0 : ts0 + 512])
                rt = work.tile([128, 512], f32, name="rtmp", bufs=2)
                nc.vector.tensor_mul(
                    rt[0:64, :], src[64:128, :], sin_sb[64:128, ts0 : ts0 + 512]
                )
                nc.vector.tensor_mul(
                    rt[64:128, :], src[0:64, :], sin_sb[0:64, ts0 : ts0 + 512]
                )
                nc.vector.tensor_add(r(dst), dst, rt)

            pending_oproj = None
            for tb in range(tb_n):
                ts0 = tb * 512
                # ============ phase A: projections + RoPE for this t-block
                qt_ps = [bank(f"qt_ps{h}") for h in range(NH_C)]
                kt_ps = bank("kt_ps")
                vt_ps = bank("vt_ps")
                for chunk in range(nd // 2):
                    c2 = 2 * chunk
                    xt = xin.tile([128, 2, 512], f32, name="xt")
                    nc.sync.dma_start(
                        out=r(xt),
                        in_=r(x_re[:, c2 : c2 + 2, ts0 : ts0 + 512]),
                    )
                    if tb == 0:
                        wq_q.dma_start(
                            out=r(wq_sb[:, c2 : c2 + 2, :]),
                            in_=r(wq_re[:, c2 : c2 + 2, :]),
                        )
                        wkv_q.dma_start(
                            out=r(wk_sb[:, c2 : c2 + 2, :]),
                            in_=r(wk_re[:, c2 : c2 + 2, :]),
                        )
                        wkv_q.dma_start(
                            out=r(wv_sb[:, c2 : c2 + 2, :]),
                            in_=r(wv_re[:, c2 : c2 + 2, :]),
                        )
                        if chunk == 0:
                            wkv_q.dma_start(out=bq_sb, in_=bqd[:, :])
                            wkv_q.dma_start(out=bk_sb, in_=bkd[:, :])
                            wkv_q.dma_start(out=bv_sb, in_=bvd[:, :])
                            nc.vector.memset(ones_f, 1.0)
                            nc.scalar.copy(out=r(ones_sb), in_=ones_f)
                    for j in range(2):
                        dt = c2 + j
                        first, last = dt == 0, dt == nd - 1
                        for h in range(NH_C):
                            nc.tensor.matmul(
                                qt_ps[h],
                                lhsT=r(wq_sb[:, dt, h * HS : (h + 1) * HS]),
                                rhs=r(xt[:, j, :]),
                                start=first,
                                stop=last,
                            )
                        nc.tensor.matmul(
                            kt_ps, lhsT=r(wk_sb[:, dt, :]), rhs=r(xt[:, j, :]),
                            start=first, stop=last,
                        )
                        nc.tensor.matmul(
                            vt_ps, lhsT=r(wv_sb[:, dt, :]), rhs=r(xt[:, j, :]),
                            start=first, stop=last,
                        )

                if tb == 0:
                    # one-time loads go on the gpsimd queue so the sync queue
                    # stays dedicated to the xt stream
                    nc.gpsimd.dma_start(out=cos_sb, in_=cosd[:, :])
                    nc.gpsimd.dma_start(out=sin_sb, in_=sind[:, :])
                    nc.gpsimd.dma_start(out=mask_sb, in_=maskd[:, :])
                    nc.gpsimd.dma_start(out=id_sb, in_=identd[:, :])
                    wo_re = wod[:, :].rearrange("(h p) m -> p h m", p=128)
                    for h in range(NH_C):
                        nc.gpsimd.dma_start(
                            out=r(wo_sb[:, h : h + 1, :]),
                            in_=r(wo_re[:, h : h + 1, :]),
                        )

                # q: bias + rope -> qf [128, h, 512]
                qf = qfp.tile([128, NH_C, 512], f32, name="qf")
                for h in range(NH_C):
                    qraw = work.tile([128, 512], f32, name="qraw")
                    nc.scalar.activation(
                        out=r(qraw), in_=qt_ps[h], func=ident_f,
                        bias=bq_sb[:, h : h + 1], scale=1.0,
                    )
                    rope(qf[:, h, :], qraw, ts0)

                # k: bias + rope -> kt_sb slice
                kraw = work.tile([128, 512], f32, name="qraw")
                nc.scalar.activation(
                    out=r(kraw), in_=kt_ps, func=ident_f, bias=bk_sb[:, 0:1],
                    scale=1.0,
                )
                rope(kt_sb[:, ts0 : ts0 + 512], kraw, ts0)

                # v: bias, then transpose to [t, hs] tiles
                vraw = work.tile([128, 512], f32, name="qraw")
                nc.scalar.activation(
                    out=vraw, in_=vt_ps, func=ident_f, bias=bv_sb[:, 0:1], scale=1.0
                )
                for s in range(4):
                    vt_tp = bank("vt_tp")
                    nc.tensor.transpose(
                        vt_tp[:, 0:128], vraw[:, 128 * s : 128 * (s + 1)], id_sb[:, :]
                    )
                    nc.scalar.copy(
                        out=r(v_sb[:, 4 * tb + s, :]), in_=vt_tp[:, 0:128]
                    )

                # previous block's o_proj: PE chews on it while ACT/DVE
                # finish this block's RoPE chain (qf not needed yet).
                if pending_oproj is not None:
                    emit_oproj(tb - 1, pending_oproj)

                # ============ phase B: attention for q-block jq == tb
                # Software-pipelined by one k-tile: PV of tile i-1 issues
                # while ACT computes exp of tile i, so the PE never waits on
                # the st->exp->mask chain.  The softmax denominator runs off
                # the PE: acc[h] += pt on DVE (pair head 0) / GpSimd (head 1).
                ot_sb = {}
                imax = 4 * tb + 3
                for hp in range(NH_C // 2):
                    heads = (2 * hp, 2 * hp + 1)
                    ot_ps = {h: bank(f"ot_ps{h}") for h in heads}
                    den_ps = {}
                    acc = {}
                    if den_off:
                        acc = {
                            h: accp.tile([128, 512], f32, name=f"acc{h & 1}")
                            for h in heads
                        }
                    else:
                        den_ps = {h: bank(f"den_ps{h}") for h in heads}

                    def emit_pv(i, pts, c0):
                        first, last = i == 0, i == imax
                        for h in heads:
                            nc.tensor.matmul(
                                ot_ps[h][:, c0:],
                                lhsT=r(v_sb[:, i, :]),
                                rhs=r(pts[h][:, c0:]),
                                start=first, stop=last,
                            )
                            if not den_off:
                                nc.tensor.matmul(
                                    den_ps[h][0:1, c0:],
                                    lhsT=r(ones_sb[:, 0:1]),
                                    rhs=r(pts[h][:, c0:]),
                                    start=first, stop=last,
                                    skip_group_check=True,
                                )

                    prev = None
                    for i in range(imax + 1):
                        c0 = 128 * max(0, i - 4 * tb)
                        diag = i >= 4 * tb
                        pts = {}
                        for h in heads:
                            st_ps = bank("st_ps")
                            nc.tensor.matmul(
                                st_ps[:, c0:],
                                lhsT=r(kt_sb[:, 128 * i : 128 * (i + 1)]),
                                rhs=r(qf[:, h, c0:]),
                                start=True, stop=True,
                            )
                            pt = work.tile([128, 512], f32, name="pt", bufs=4)
                            nc.scalar.activation(
                                out=r(pt[:, c0:]), in_=st_ps[:, c0:], func=exp_f,
                                scale=INV_SQRT_HS,
                            )
                            if diag:
                                nc.vector.tensor_mul(
                                    r(pt[:, c0 : c0 + 128]),
                                    pt[:, c0 : c0 + 128],
                                    mask_sb,
                                )
                            pts[h] = pt
                            if den_off:
                                # engine-local accumulation chain per head
                                eng = nc.vector if (h & 1) == 0 else nc.gpsimd
                                if i == 0:
                                    eng.tensor_copy(
                                        out=r(acc[h][:, c0:]), in_=pt[:, c0:]
                                    )
                                else:
                                    eng.tensor_add(
                                        r(acc[h][:, c0:]), acc[h][:, c0:], pt[:, c0:]
                                    )
                        if prev is not None:
                            emit_pv(*prev)
                        prev = (i, pts, c0)
                    emit_pv(*prev)

                    # normalize each head's O^T by its softmax denominator:
                    # one ones^T @ acc matmul makes the [1,512] denominator
                    # row; reciprocal_approx_fast on that row (DVE, ~18-bit),
                    # GpSimd partition_broadcast fans it out to [128,512], and
                    # DVE multiplies it into the evacuated O^T.  Evacuate O^T
                    # (ACT copies) immediately so PSUM banks free early.
                    osb_h = {}
                    denrow = {}
                    for h in heads:
                        osb = otp.tile([128, 512], f32, name="osb", bufs=8)
                        nc.scalar.copy(out=r(osb), in_=ot_ps[h])
                        osb_h[h] = osb
                        if den_off:
                            dps = bank("den_ps")
                            nc.tensor.matmul(
                                dps[0:1, :],
                                lhsT=r(ones_sb[:, 0:1]),
                                rhs=r(acc[h]),
                                start=True, stop=True,
                                skip_group_check=True,
                            )
                            den_ps[h] = dps
                        dr = work.tile([1, 512], f32, name="den_sb", bufs=2)
                        nc.scalar.copy(out=dr, in_=den_ps[h][0:1, :])
                        denrow[h] = dr
                    for h in heads:
                        bc_sb = work.tile([128, 512], f32, name="bc_sb", bufs=2)
                        if pbcast:
                            nc.vector.reciprocal_approx_fast(
                                out=denrow[h], in_=denrow[h]
                            )
                            nc.gpsimd.partition_broadcast(
                                bc_sb, denrow[h], channels=128
                            )
                        else:
                            bc_ps = bank("bc_ps")
                            nc.tensor.matmul(
                                bc_ps,
                                lhsT=r(ones_sb[0:1, 0:128]),
                                rhs=r(denrow[h]),
                                start=True, stop=True,
                            )
                            nc.scalar.copy(out=bc_sb, in_=bc_ps)
                            nc.vector.reciprocal_approx_fast(out=bc_sb, in_=bc_sb)
                        nc.vector.tensor_mul(r(osb_h[h]), osb_h[h], bc_sb)
                        ot_sb[h] = osb_h[h]

                pending_oproj = ot_sb

            emit_oproj(tb_n - 1, pending_oproj)

    nc.compile()
    return nc


def shard_inputs(x, cos, sin, Wq, bq, Wkv, bkv, Wo, t=T):
    """Build the 8 per-core input maps (core c -> batch c//4, group c%4)."""
    f32 = np.float32
    mask_ut = np.triu(np.ones((128, 128), f32))
    ident = np.eye(128, dtype=f32)
    cos_t = np.ascontiguousarray(cos.T.astype(f32))
    # sin_sw: halves swapped with the rotate-half sign folded in, so the
    # on-chip rotate is two same-base-partition multiplies plus an add:
    # sin_sw[p<64] = +sin^T[p+64], sin_sw[p>=64] = -sin^T[p-64].
    st = sin.T.astype(f32)
    sin_t = np.ascontiguousarray(np.concatenate([st[64:128], -st[0:64]], axis=0))

    xts = [np.ascontiguousarray(x[b].T.astype(f32)) for b in range(x.shape[0])]
    per_g = []
    for g in range(4):
        per_g.append(
            dict(
                wq_t=np.ascontiguousarray(Wq[512 * g : 512 * g + 512].T.astype(f32)),
                b_q=np.ascontiguousarray(
                    bq[512 * g : 512 * g + 512].reshape(4, 128).T.astype(f32)
                ),
                wk_t=np.ascontiguousarray(
                    Wkv[128 * g : 128 * g + 128].T.astype(f32)
                ),
                b_k=np.ascontiguousarray(
                    bkv[128 * g : 128 * g + 128].reshape(128, 1).astype(f32)
                ),
                wv_t=np.ascontiguousarray(
                    Wkv[512 + 128 * g : 512 + 128 * g + 128].T.astype(f32)
                ),
                b_v=np.ascontiguousarray(
                    bkv[512 + 128 * g : 512 + 128 * g + 128]
                    .reshape(128, 1)
                    .astype(f32)
                ),
                wo_t=np.ascontiguousarray(
                    Wo[:, 512 * g : 512 * g + 512].T.astype(f32)
                ),
            )
        )

    in_maps = []
    for c in range(4 * x.shape[0]):
        b, g = c // 4, c % 4
        m = dict(per_g[g])
        m.update(
            x_t=xts[b], cos_t=cos_t, sin_t=sin_t,
            mask_ut=mask_ut, ident=ident,
        )
        in_maps.append(m)
    return in_maps


def run_on_hw(in_maps, t=T, trace=False, mm_r=True, **flags):
    from concourse.bass_utils import run_bass_kernel_spmd

    key = (t, mm_r, tuple(sorted(flags.items())))
    if key not in _NC_CACHE:
        _NC_CACHE[key] = build_nc(t, mm_r=mm_r, **flags)
    nc = _NC_CACHE[key]
    res = run_bass_kernel_spmd(
        nc, in_maps, core_ids=list(range(len(in_maps))), trace=trace
    )
    return res


def kernel(x, cos, sin, Wq, bq, Wkv, bkv, Wo):
    x = np.asarray(x)
    in_maps = shard_inputs(
        x, np.asarray(cos), np.asarray(sin), np.asarray(Wq), np.asarray(bq),
        np.asarray(Wkv), np.asarray(bkv), np.asarray(Wo),
    )
    res = run_on_hw(in_maps, t=T, trace=False)
    out = np.zeros((B, T, D), np.float32)
    for c, rmap in enumerate(res.results):
        out[c // 4] += np.asarray(rmap["out"], dtype=np.float32)
    return out
